# revision 35
# baseline (speedup 1.0000x reference)
"""Decoder block Bass/Tile kernel for TRN2, SPMD over 8 cores — head-parallel.

Sharding (Plan E): core c owns head-pair hp=c (heads 2c, 2c+1) for BOTH
batches, computing QKV + causal attention for all 4096 queries of its pair
with exact per-chunk key lengths (q-chunk ci of 256 needs exactly 2(ci+1)
k-tiles — identical on every core, so the single SPMD program is uniform
with zero padding waste). Two per-batch 8-core HBM AllToAlls (512KB each)
reshard attention output from heads to tokens — batch 0's fires as soon as
its attention drains and hides under batch 1's attention. Core c then owns
the 256-token quarter [256c, 256c+256) of EACH batch for proj + residual +
LN2 + MLP.

Everything runs in "fm" layout ([feature(partition), token(free)]).
LayerNorm statistics on the PE (ones-column matmuls), rsqrt via ACT Ln+Exp,
per-token (mean,rstd) broadcast by outer-product matmuls, applied by DVE in
fp16 2x mode. Scores batch 2 k-tiles per PSUM mega-tile; exp on ACT; the
two diagonal k-tiles of each q-chunk are masked by a 0/1 multiply on pt
AFTER exp (keeps DVE off the PE->ACT exp chain — the AV matmul absorbs the
wait). Softmax denominators ride as constant-1 columns of V; at eviction
rec = exp(-ln(den)) on ACT is PE-broadcast into the unused upper partitions
of the same PSUM bank and divided out by DVE. MLP weight streams are
emitted only up to what their pools can buffer (a dma_start stalled on a
pool buffer blocks everything behind it on that engine queue); the tail
streams through a wide second pool across both DMA queues once the
attention tiles are dead. Matmuls fp16 with fp32 PSUM accumulation;
residual stream fp32.
"""

from contextlib import ExitStack
from dataclasses import dataclass

import numpy as np

import concourse.bass as bass
import concourse.tile as tile
from concourse import mybir
from concourse._compat import with_exitstack

F32 = mybir.dt.float32
F16 = mybir.dt.float16
MASK_NEG = -60000.0
AF = mybir.ActivationFunctionType
A2A_GROUPS = [[0, 1, 2, 3, 4, 5, 6, 7]]


@dataclass
class Cfg:
    D: int = 1024
    DFF: int = 4096
    H: int = 16
    DH: int = 64
    T: int = 2048
    B: int = 2
    CH: int = 256
    mmdt: str = "float16"

    @property
    def ND(self):
        return self.D // 128

    @property
    def NFF(self):
        return self.DFF // 128

    @property
    def NQC(self):  # q-chunks of 256 per batch
        return self.T // self.CH

    @property
    def NTKV(self):  # k-tiles of 128 per batch
        return self.T // 128


def _bcast_ap(ap, p=128):
    """[N] dram AP -> [p, N] with partition stride 0."""
    return bass.AP(tensor=ap.tensor, offset=ap.offset, ap=[[0, p]] + list(ap.ap))


def _groups(nkt, w):
    return [(g0, min(g0 + w, nkt)) for g0 in range(0, nkt, w)]


@with_exitstack
def decoder_kernel(ctx: ExitStack, tc: tile.TileContext, cfg: Cfg, io: dict):
    nc = tc.nc
    MD = getattr(mybir.dt, cfg.mmdt)
    D, CH = cfg.D, cfg.CH
    ND, NFF, NQC, NTKV = cfg.ND, cfg.NFF, cfg.NQC, cfg.NTKV
    B, T = cfg.B, cfg.T
    NCH = B * T // 512  # 512-token x chunks across both batches

    # ---------------- constants ----------------
    const = ctx.enter_context(tc.tile_pool(name="const", bufs=1))
    eps_t = const.tile([1, 1], F32)
    nc.vector.memset(eps_t, 1e-5)
    ones_col = const.tile([128, 1], MD)
    nc.vector.memset(ones_col, 1.0)
    ones_row = const.tile([1, 128], MD)
    nc.vector.memset(ones_row, 1.0)
    neg_row = const.tile([1, 128], MD)
    nc.vector.memset(neg_row, -1.0)
    bq_sb = const.tile([128, 1], F32)
    nc.sync.dma_start(out=bq_sb, in_=io["bq"].rearrange("(t p) -> p t", p=128))
    bk_sb = const.tile([128, 1], F32)
    nc.sync.dma_start(out=bk_sb, in_=io["bk"].rearrange("(t p) -> p t", p=128))
    bfc1_sb = const.tile([128, NFF], F32)
    nc.sync.dma_start(out=bfc1_sb, in_=io["bfc1"].rearrange("(t p) -> p t", p=128))
    vb_sb = const.tile([128, 128], F32)
    nc.sync.dma_start(out=vb_sb, in_=_bcast_ap(io["vb"]))
    maskd = const.tile([128, 512], MD)  # two diagonal mask tiles [128,256]
    nc.sync.dma_start(out=maskd, in_=io["maskd"])
    wk_t = const.tile([128, ND, 128], MD)
    nc.sync.dma_start(out=wk_t, in_=io["wk"])
    wq_t = const.tile([128, ND, 128], MD)
    nc.sync.dma_start(out=wq_t, in_=io["wq"])
    wv_sb = const.tile([128, ND, 128], MD)
    nc.sync.dma_start(out=wv_sb, in_=io["wv"])
    ckn_sb = const.tile([1, 128], MD)
    nc.sync.dma_start(out=ckn_sb, in_=io["ckn"])
    cqn_sb = const.tile([1, 128], MD)
    nc.sync.dma_start(out=cqn_sb, in_=io["cqn"])
    cvn_sb = const.tile([1, 128], MD)
    nc.sync.dma_start(out=cvn_sb, in_=io["cvn"])

    # dram bounce for the per-batch AllToAlls (batch 0's reshard fires as
    # soon as batch 0's attention drains, hiding under batch 1's attention)
    dram = ctx.enter_context(tc.tile_pool(name="dram", bufs=1, space="DRAM"))
    a2a_in = [dram.tile([1024, 256], MD, name=f"a2ai{b}") for b in range(2)]
    a2a_out = [dram.tile([1024, 256], MD, name=f"a2ao{b}") for b in range(2)]

    with tc.tile_pool(name="kqv_acts", bufs=1) as acts:
        K_sb = acts.tile([128, B * T], MD, name="K")
        Qp = acts.tile([128, 2 * B * T], MD, name="Qp")
        nc.vector.memset(Qp, 0.0)
        V_sb = [acts.tile([128, 130], MD, tag=f"V{t}", name=f"V{t}")
                for t in range(B * NTKV)]
        for t in range(B * NTKV):  # constant denominator columns (one per head)
            nc.vector.memset(
                V_sb[t].rearrange("p (b c) -> p b c", c=65)[:, :, 64:65], 1.0
            )
        O_sb = acts.tile([128, B * T], MD, name="O")
        XQ32 = [acts.tile([128, 512], F32, tag=f"XQ32_{d}", name=f"XQ32_{d}")
                for d in range(ND)]

        # right-side pools: weight prefetch + fp32 state (live to the end)
        awp = ctx.enter_context(tc.tile_pool(name="attw", bufs=1, side="right"))
        f1w = ctx.enter_context(tc.tile_pool(name="mlpw1", bufs=23, side="right"))
        f2w = ctx.enter_context(tc.tile_pool(name="mlpw2", bufs=12, side="right"))
        x2P = ctx.enter_context(tc.tile_pool(name="x2P", bufs=1, side="right"))
        x2cP = ctx.enter_context(tc.tile_pool(name="x2cP", bufs=1, side="right"))

        # =================== phase A: LN1 + QKV ===================
        with tc.tile_pool(name="xr", bufs=3) as xrp, tc.tile_pool(
            name="xsq", bufs=1
        ) as xsqp, tc.tile_pool(name="lnsm", bufs=1) as lnsm, tc.tile_pool(
            name="stps", bufs=1, space="PSUM"
        ) as stps, tc.tile_pool(name="bcps", bufs=1, space="PSUM") as bcps, \
            tc.tile_pool(name="acps", bufs=1, space="PSUM") as acps, \
            tc.tile_pool(name="qkvps", bufs=1, space="PSUM") as qkvps:

            def ln_stats(xtiles, sqtiles, tag):
                """Per-token LN stats. Returns ([1,512] fp16 rstd, mean*rstd)."""
                ps_s = stps.tile([1, 512], F32, tag="ps_s", name=f"ps_s{tag}")
                ps_q = stps.tile([1, 512], F32, tag="ps_q", name=f"ps_q{tag}")
                for d in range(ND):
                    nc.tensor.matmul(ps_s, ones_col, xtiles[d],
                                     start=(d == 0), stop=(d == ND - 1))
                for d in range(ND):
                    nc.tensor.matmul(ps_q, ones_col, sqtiles[d],
                                     start=(d == 0), stop=(d == ND - 1))
                mu = lnsm.tile([1, 512], F32, tag="mu", name=f"mu{tag}")
                nc.vector.tensor_scalar_mul(out=mu, in0=ps_s, scalar1=1.0 / D)
                msq = lnsm.tile([1, 512], F32, tag="msq", name=f"msq{tag}")
                nc.vector.tensor_scalar_mul(out=msq, in0=ps_q, scalar1=1.0 / D)
                nc.vector.tensor_mul(out=ps_s, in0=mu, in1=mu)
                nc.vector.tensor_sub(out=msq, in0=msq, in1=ps_s)
                nc.scalar.activation(out=ps_q, in_=msq, func=AF.Ln, bias=eps_t)
                a16 = lnsm.tile([1, 512], MD, tag="a16", name=f"a16{tag}",
                                bufs=2)
                nc.scalar.activation(out=a16, in_=ps_q, func=AF.Exp, scale=-0.5)
                mu16 = lnsm.tile([1, 512], MD, tag="mu16", name=f"mu16{tag}",
                                 bufs=2)
                nc.vector.tensor_copy(out=mu16, in_=mu)
                return a16, mu16

            def ln_prep(a16, tag):
                """Broadcasts of rstd for the projection-side LN fold:
                a_sb [128,512] (rstd per token column, fp16) and acol
                [128,4] (rstd per token PARTITION, for V's layout)."""
                a_bc = bcps.tile([128, 512], F32, tag="abc", name=f"abc{tag}")
                nc.tensor.matmul(a_bc, ones_row, a16, start=True, stop=True)
                a_sb = lnsm.tile([128, 512], MD, tag="asb", name=f"asb{tag}",
                                 bufs=2)
                nc.scalar.copy(out=a_sb, in_=a_bc)
                acol = acps.tile([128, 4], F32, tag="acol", name=f"acol{tag}",
                                 bufs=2)
                for j in range(4):
                    nc.tensor.matmul(
                        acol[:, j : j + 1],
                        a16[0:1, j * 128 : (j + 1) * 128],
                        ones_row[0:1, 0:1],
                        start=True, stop=True,
                    )
                return a_sb, acol

            def load_chunk(c):
                xr = [xrp.tile([128, 512], MD, tag=f"xr{d}", name=f"xr{c}_{d}")
                      for d in range(ND)]
                for d in range(ND):
                    nc.gpsimd.dma_start(
                        out=xr[d],
                        in_=io["x_fm"][d * 128 : (d + 1) * 128,
                                       c * 512 : (c + 1) * 512],
                    )
                sq = [xsqp.tile([128, 512], MD, tag=f"xsq{d}", name=f"sq{c}_{d}")
                      for d in range(ND)]
                for d in range(ND):
                    nc.scalar.activation(out=sq[d], in_=xr[d], func=AF.Square)
                return xr, sq

            # Projections run on RAW x; LayerNorm folds in afterward:
            #   P_ln = a_t * (P_raw + mu_t * (-col_sum(W))) + bias
            # so the x matmuls start straight off the DMA, and only the
            # final rank-1 accumulate + eviction wait on the LN stats.
            def v_proj(c, xp, mu16, acol):
                for j in range(4):
                    vt = 4 * c + j
                    ps = qkvps.tile([128, 128], F32, tag="vps", bufs=1,
                                    name=f"vps{vt}")
                    for d in range(ND):
                        nc.tensor.matmul(
                            ps, xp[d][:, j * 128 : (j + 1) * 128],
                            wv_sb[:, d, :],
                            start=(d == 0), stop=False,
                        )
                    nc.tensor.matmul(
                        ps, mu16[0:1, j * 128 : (j + 1) * 128], cvn_sb,
                        start=False, stop=True,
                    )
                    vt16 = lnsm.tile([128, 128], MD, tag="vt16",
                                     name=f"vt16_{vt}")
                    nc.vector.tensor_scalar_mul(out=vt16, in0=ps,
                                                scalar1=acol[:, j : j + 1])
                    dst = V_sb[vt].rearrange("p (b c) -> p b c", c=65)[:, :, 0:64]
                    nc.vector.tensor_add(
                        out=dst,
                        in0=vt16.rearrange("p (b c) -> p b c", c=64),
                        in1=vb_sb.rearrange("p (b c) -> p b c", c=64),
                    )

            def k_proj(c, xp, a_sb, mu16):
                ps = qkvps.tile([128, 512], F32, tag="kps", bufs=1,
                                name=f"kps{c}")
                for d in range(ND):
                    nc.tensor.matmul(ps, wk_t[:, d, :], xp[d],
                                     start=(d == 0), stop=False)
                nc.tensor.matmul(ps, ckn_sb, mu16, start=False, stop=True)
                ksl = K_sb[:, c * 512 : (c + 1) * 512]
                nc.vector.tensor_mul(out=ksl, in0=ps, in1=a_sb)
                nc.vector.tensor_scalar_add(out=ksl, in0=ksl,
                                            scalar1=bk_sb[:, 0:1])

            def q_proj(c, xp, a_sb, mu16):
                b, lc = c // 4, c % 4
                ps = qkvps.tile([128, 512], F32, tag="qps", bufs=1,
                                name=f"qps{c}")
                for d in range(ND):
                    nc.tensor.matmul(ps, wq_t[:, d, :], xp[d],
                                     start=(d == 0), stop=False)
                nc.tensor.matmul(ps, cqn_sb, mu16, start=False, stop=True)
                qt = lnsm.tile([128, 512], MD, tag="qt16", name=f"qt16_{c}")
                nc.vector.tensor_mul(out=qt, in0=ps, in1=a_sb)
                for sub in range(2):
                    ci = 2 * lc + sub
                    for h in range(2):
                        base = b * 2 * T + ci * 512 + h * CH
                        nc.vector.tensor_scalar_add(
                            out=Qp[h * 64 : (h + 1) * 64, base : base + CH],
                            in0=qt[h * 64 : (h + 1) * 64,
                                   sub * CH : (sub + 1) * CH],
                            scalar1=bq_sb[h * 64 : (h + 1) * 64, 0:1],
                        )

            # 2-deep chunk pipeline: while chunk c's projections run on the
            # PE, chunk c+1 is in its LN-stats chain and chunk c+2 is loading
            chunks = {0: load_chunk(0)}
            a0, mu0 = ln_stats(*chunks[0], "c0")
            prep = {0: (ln_prep(a0, "c0"), mu0)}
            chunks[1] = load_chunk(1)
            for c in range(NCH):
                if c + 2 < NCH:
                    chunks[c + 2] = load_chunk(c + 2)
                if c + 1 < NCH:
                    a1, mu1 = ln_stats(*chunks[c + 1], f"c{c+1}")
                    prep[c + 1] = (ln_prep(a1, f"c{c+1}"), mu1)
                xp_c = chunks.pop(c)[0]
                (a_sb_c, acol_c), mu_c = prep.pop(c)
                v_proj(c, xp_c, mu_c, acol_c)
                k_proj(c, xp_c, a_sb_c, mu_c)
                q_proj(c, xp_c, a_sb_c, mu_c)
            # residual block for phase D — needed only after the a2a, so
            # load behind the x chunks on the same queue
            for d in range(ND):
                nc.gpsimd.dma_start(
                    out=XQ32[d], in_=io["xq32"][d * 128 : (d + 1) * 128, :]
                )

        # prefetch proj + MLP weights on the sync queue, but ONLY as many as
        # the pools can buffer — a dma_start stalled on a pool buffer would
        # block everything emitted after it on the same engine (v1 lost 38µs
        # at the a2a barrier exactly this way). The rest are emitted after
        # the a2a dumps.
        wproj_sb = [awp.tile([128, D], MD, tag=f"wp{r}", name=f"wp{r}")
                    for r in range(ND)]
        for r in range(ND):
            nc.sync.dma_start(
                out=wproj_sb[r], in_=io["wproj"][r * 128 : (r + 1) * 128, :]
            )
        w1t = []
        for ff in range(23):
            t = f1w.tile([128, ND, 128], MD, tag="wfc1", name=f"wfc1_{ff}")
            nc.sync.dma_start(out=t, in_=io["wfc1"][ff])
            w1t.append(t)
        w2t = {}
        for ff in range(12):
            t = f2w.tile([128, 512], MD, tag="wfc2", name=f"wfc2_0_{ff}")
            nc.sync.dma_start(out=t, in_=io["wfc2"][0, ff])
            w2t[(0, ff)] = t

        # =================== phase B: attention ===================
        with tc.tile_pool(name="attpt", bufs=4) as ptp, tc.tile_pool(
            name="attsm", bufs=2
        ) as smp, tc.tile_pool(name="scps", bufs=3, space="PSUM") as scps, \
                tc.tile_pool(name="pops", bufs=2, space="PSUM") as pops:

            units = []  # (b, ci, g0, g1, nkt)
            for b in range(B):
                for ci in range(NQC):
                    nkt = 2 * (ci + 1)
                    for g0, g1 in _groups(nkt, 2):
                        units.append((b, ci, g0, g1, nkt))

            po_of = {}
            pend = []

            def emit_S(u):
                b, ci, g0, g1, nkt = u
                w = (g1 - g0) * 512
                sc = scps.tile([128, 1024], F32, tag="sc",
                               name=f"sc{b}_{ci}_{g0}")
                for k in range(g0, g1):
                    nc.tensor.matmul(
                        sc[:, (k - g0) * 512 : (k - g0 + 1) * 512],
                        K_sb[:, (b * NTKV + k) * 128 : (b * NTKV + k + 1) * 128],
                        Qp[:, b * 2 * T + ci * 512 : b * 2 * T + (ci + 1) * 512],
                        start=True, stop=True,
                    )
                pt = ptp.tile([128, 1024], MD, tag="pt",
                              name=f"pt{b}_{ci}_{g0}")
                nc.scalar.activation(out=pt[:, 0:w], in_=sc[:, 0:w], func=AF.Exp)
                # causal mask as a 0/1 multiply on pt AFTER exp: keeps DVE off
                # the PE->ACT exp chain (the AV matmul absorbs the wait)
                for k in range(max(g0, nkt - 2), g1):
                    di = k - (nkt - 2)  # 0 or 1: which diagonal mask tile
                    ptv = pt[:, (k - g0) * 512 : (k - g0 + 1) * 512].rearrange(
                        "p (h q) -> p h q", q=256
                    )
                    msl = maskd[:, di * 256 : (di + 1) * 256]
                    mkb = bass.AP(
                        tensor=msl.tensor, offset=msl.offset,
                        ap=[list(msl.ap[0]), [0, 2], [1, 256]],
                    )
                    nc.vector.tensor_mul(out=ptv, in0=ptv, in1=mkb)
                pend.append((pt, u))

            def emit_AV():
                pt, (b, ci, g0, g1, nkt) = pend.pop(0)
                if g0 == 0:
                    po_of[(b, ci)] = pops.tile([128, 512], F32, tag="po",
                                               name=f"po{b}_{ci}")
                po = po_of[(b, ci)]
                for k in range(g0, g1):
                    for h in range(2):
                        # h0+h1 are ONE accumulation group (start resets the
                        # whole bank's has_written)
                        nc.tensor.matmul(
                            po[0:65, h * CH : (h + 1) * CH],
                            V_sb[b * NTKV + k][:, h * 65 : h * 65 + 65],
                            pt[:, (k - g0) * 512 + h * CH :
                               (k - g0) * 512 + (h + 1) * CH],
                            start=(k == 0 and h == 0),
                            stop=(k == nkt - 1 and h == 1),
                            skip_group_check=True,
                        )
                return (b, ci) if g1 == nkt else None

            def drain(b, ci):
                po = po_of.pop((b, ci))
                den = smp.tile([1, 512], F32, tag="den", name=f"den{b}_{ci}")
                nc.vector.tensor_copy(out=den, in_=po[64:65, :])
                lnd = smp.tile([1, 512], F32, tag="lnd", name=f"lnd{b}_{ci}")
                nc.scalar.activation(out=lnd, in_=den, func=AF.Ln)
                rec = smp.tile([1, 512], MD, tag="rec", name=f"rec{b}_{ci}")
                nc.scalar.activation(out=rec, in_=lnd, func=AF.Exp, scale=-1.0)
                for h in range(2):
                    nc.tensor.matmul(
                        po[64:128, h * CH : (h + 1) * CH],
                        ones_row[0:1, 0:64],
                        rec[0:1, h * CH : (h + 1) * CH],
                        start=True, stop=True,
                    )
                rb = smp.tile([64, 512], MD, tag="rb", name=f"rb{b}_{ci}")
                nc.vector.tensor_copy(out=rb, in_=po[64:128, :])
                for h in range(2):
                    nc.vector.tensor_mul(
                        out=O_sb[h * 64 : (h + 1) * 64,
                                 b * T + ci * CH : b * T + (ci + 1) * CH],
                        in0=po[0:64, h * CH : (h + 1) * CH],
                        in1=rb[:, h * CH : (h + 1) * CH],
                    )

            def finish(fin):
                """Drain, then immediately dump this chunk's slice of the
                a2a input (dump d depends only on drain (b, d)), so the
                collective barrier isn't gated on 8 back-to-back DMAs at
                the end. Heads -> tokens: core c ends up owning tokens
                [256c, 256c+256) of batch b."""
                drain(*fin)
                b, ci = fin
                nc.sync.dma_start(
                    out=a2a_in[b][ci * 128 : (ci + 1) * 128, :],
                    in_=O_sb[:, b * T + ci * 256 : b * T + (ci + 1) * 256],
                )
                if ci == NQC - 1:
                    nc.gpsimd.collective_compute(
                        "AllToAll",
                        mybir.AluOpType.bypass,
                        replica_groups=A2A_GROUPS,
                        ins=[a2a_in[b].opt()],
                        outs=[a2a_out[b].opt()],
                    )

            for i, u in enumerate(units):
                emit_S(u)
                if i >= 3:
                    fin = emit_AV()
                    if fin is not None:
                        finish(fin)
            for _ in range(3):
                fin = emit_AV()
                if fin is not None:
                    finish(fin)
        # tail of the wfc1 stream — on gpsimd so its pool-buffer stalls can't
        # block the wfc2 stream behind it on sync
        for ff in range(23, NFF):
            t = f1w.tile([128, ND, 128], MD, tag="wfc1", name=f"wfc1_{ff}")
            nc.gpsimd.dma_start(out=t, in_=io["wfc1"][ff])
            w1t.append(t)
        x2_sb = [x2P.tile([128, 512], F32, tag=f"x2_{d}", name=f"x2_{d}")
                 for d in range(ND)]
        X2c = [x2cP.tile([128, 512], MD, tag=f"x2c{d}", name=f"X2c{d}")
               for d in range(ND)]

        # ============ phase D: proj + residual + LN2 ============
        with tc.tile_pool(name="prj", bufs=1) as prj, tc.tile_pool(
            name="prps", bufs=4, space="PSUM"
        ) as prps, tc.tile_pool(name="ln2sm", bufs=1) as ln2sm, tc.tile_pool(
            name="st2ps", bufs=1, space="PSUM"
        ) as st2ps, tc.tile_pool(name="bc2ps", bufs=1, space="PSUM") as bc2ps:
            P_sb = [prj.tile([128, 512], MD, tag=f"P{r}", name=f"P{r}")
                    for r in range(ND)]
            for r in range(ND):
                for b in range(2):
                    nc.gpsimd.dma_start(
                        out=P_sb[r][:, b * 256 : (b + 1) * 256],
                        in_=a2a_out[b][r * 128 : (r + 1) * 128, :],
                    )
            x2sq = [ln2sm.tile([128, 512], MD, tag=f"x2sq{d}",
                               name=f"x2sq{d}") for d in range(ND)]
            for do in range(ND):
                pp = prps.tile([128, 512], F32, tag="pp", name=f"pp{do}")
                for r in range(ND):
                    nc.tensor.matmul(
                        pp, wproj_sb[r][:, do * 128 : (do + 1) * 128],
                        P_sb[r],
                        start=(r == 0), stop=(r == ND - 1),
                    )
                nc.vector.tensor_add(out=x2_sb[do], in0=pp, in1=XQ32[do])
                nc.vector.tensor_copy(out=X2c[do], in_=x2_sb[do])
                nc.scalar.activation(out=x2sq[do], in_=x2_sb[do],
                                     func=AF.Square)

            ps_s = st2ps.tile([1, 512], F32, tag="ps_s2")
            ps_q = st2ps.tile([1, 512], F32, tag="ps_q2")
            for d in range(ND):
                nc.tensor.matmul(ps_s, ones_col, X2c[d],
                                 start=(d == 0), stop=(d == ND - 1))
            for d in range(ND):
                nc.tensor.matmul(ps_q, ones_col, x2sq[d],
                                 start=(d == 0), stop=(d == ND - 1))
            mu = ln2sm.tile([1, 512], F32, tag="mu2")
            nc.vector.tensor_scalar_mul(out=mu, in0=ps_s, scalar1=1.0 / D)
            msq = ln2sm.tile([1, 512], F32, tag="msq2")
            nc.vector.tensor_scalar_mul(out=msq, in0=ps_q, scalar1=1.0 / D)
            nc.vector.tensor_mul(out=ps_s, in0=mu, in1=mu)
            nc.vector.tensor_sub(out=msq, in0=msq, in1=ps_s)
            nc.scalar.activation(out=ps_q, in_=msq, func=AF.Ln, bias=eps_t)
            a16 = ln2sm.tile([1, 512], MD, tag="a162")
            nc.scalar.activation(out=a16, in_=ps_q, func=AF.Exp, scale=-0.5)
            nb16 = ln2sm.tile([1, 512], MD, tag="nb162")
            nc.vector.tensor_mul(out=nb16, in0=mu, in1=a16)
            a_bc = bc2ps.tile([128, 512], F32, tag="abc2")
            nc.tensor.matmul(a_bc, ones_row, a16, start=True, stop=True)
            b_bc = bc2ps.tile([128, 512], F32, tag="bbc2")
            nc.tensor.matmul(b_bc, neg_row, nb16, start=True, stop=True)
            for d in range(ND):
                nc.vector.tensor_mul(out=X2c[d], in0=X2c[d], in1=a_bc)
                nc.vector.tensor_add(out=X2c[d], in0=X2c[d], in1=b_bc)

    # =================== phase E: MLP ===================
    # the attention tiles are dead now — a wide second wfc2 pool lets the
    # remaining 52 tiles stream without stalling behind pool-buffer waits
    f2wB = ctx.enter_context(tc.tile_pool(name="mlpw2b", bufs=40, side="right"))
    for s in range(2):
        for ff in range(12 if s == 0 else 0, NFF):
            t = f2wB.tile([128, 512], MD, tag="wfc2b", name=f"wfc2b_{s}_{ff}")
            eng = nc.sync if ff % 2 == 0 else nc.gpsimd
            eng.dma_start(out=t, in_=io["wfc2"][s, ff])
            w2t[(s, ff)] = t
    with tc.tile_pool(name="gh", bufs=1) as ghp, tc.tile_pool(
        name="ostg", bufs=2
    ) as ostg, tc.tile_pool(name="f1ps", bufs=4, space="PSUM") as fps, \
            tc.tile_pool(name="accps", bufs=1, space="PSUM") as aps:
        gh_sb = [ghp.tile([128, 512], MD, tag=f"gh{f}", name=f"gh{f}")
                 for f in range(NFF)]
        acc = [aps.tile([128, 512], F32, tag=f"acc{dt}", name=f"acc{dt}")
               for dt in range(4)]

        def fc1(ff):
            ps1 = fps.tile([128, 512], F32, tag="ps1", name=f"ps1_{ff}")
            for d in range(ND):
                nc.tensor.matmul(ps1, w1t[ff][:, d, :], X2c[d],
                                 start=(d == 0), stop=(d == ND - 1))
            nc.scalar.activation(out=gh_sb[ff], in_=ps1, func=AF.Gelu,
                                 bias=bfc1_sb[:, ff : ff + 1])

        def fc2(s, ff, accs):
            for dt in range(4):
                nc.tensor.matmul(
                    accs[dt],
                    w2t[(s, ff)][:, dt * 128 : (dt + 1) * 128],
                    gh_sb[ff],
                    start=(ff == 0), stop=(ff == NFF - 1),
                )

        fc1(0)
        for ff in range(1, NFF):
            fc1(ff)
            fc2(0, ff - 1, acc)
        fc2(0, NFF - 1, acc)
        for dt in range(4):
            o = ostg.tile([128, 512], F32, tag="ostg", name=f"o{dt}")
            nc.vector.tensor_add(out=o, in0=acc[dt], in1=x2_sb[dt])
            eng = nc.gpsimd if dt % 2 == 0 else nc.sync
            eng.dma_start(out=io["out"][dt * 128 : (dt + 1) * 128, :], in_=o)
        acc2 = [aps.tile([128, 512], F32, tag=f"acc{dt}", name=f"acc2_{dt}")
                for dt in range(4)]
        for ff in range(NFF - 1):
            fc2(1, ff, acc2)
        for dt in range(4):
            nc.tensor.matmul(
                acc2[dt],
                w2t[(1, NFF - 1)][:, dt * 128 : (dt + 1) * 128],
                gh_sb[NFF - 1],
                start=False, stop=True,
            )
            o = ostg.tile([128, 512], F32, tag="ostg", name=f"o2_{dt}")
            nc.vector.tensor_add(out=o, in0=acc2[dt], in1=x2_sb[4 + dt])
            eng = nc.gpsimd if dt % 2 == 0 else nc.sync
            eng.dma_start(
                out=io["out"][(4 + dt) * 128 : (4 + dt + 1) * 128, :], in_=o
            )


def split_drain_waits(nc):
    """walrus CoreV3 rejects >1 sync wait on several instruction types;
    split extras into single-wait NOPs preceding the instruction on the
    same (in-order) engine."""
    idx = 0

    def fix_block(b):
        nonlocal idx
        new = []
        changed = False
        for inst in b.instructions:
            si = inst.sync_info
            if si is not None and si.on_wait and len(si.on_wait) > 1:
                waits = list(si.on_wait)
                for w in waits[:-1]:
                    idx += 1
                    nop = mybir.InstNoOp(
                        name=f"I-dsplit-{idx}",
                        sync_info=mybir.SyncInfo(on_wait=[w], on_update=[]),
                    )
                    nop.engine = inst.engine
                    new.append(nop)
                inst.sync_info = mybir.SyncInfo(
                    on_wait=[waits[-1]], on_update=list(si.on_update or [])
                )
                changed = True
            new.append(inst)
        if changed:
            b.instructions = new

    for f in nc.m.functions:
        for b in f.blocks:
            fix_block(b)


def declare_io(nc, cfg: Cfg):
    c = cfg
    WD = getattr(mybir.dt, c.mmdt)
    ND, NFF = c.ND, c.NFF
    spec = {
        "x_fm": ([c.D, c.B * c.T], WD, False),
        "xq32": ([c.D, 512], F32, False),
        "wq": ([128, ND, 128], WD, False),
        "wk": ([128, ND, 128], WD, False),
        "wv": ([128, ND, 128], WD, False),
        "bq": ([128], F32, False),
        "bk": ([128], F32, False),
        "vb": ([128], F32, False),
        "ckn": ([1, 128], WD, False),
        "cqn": ([1, 128], WD, False),
        "cvn": ([1, 128], WD, False),
        "wproj": ([c.D, c.D], WD, False),
        "wfc1": ([NFF, 128, ND, 128], WD, False),
        "bfc1": ([c.DFF], F32, False),
        "wfc2": ([2, NFF, 128, 512], WD, False),
        "maskd": ([128, 512], WD, False),
        "out": ([c.D, 512], F32, True),
    }
    io = {}
    for name, (shape, dt, is_out) in spec.items():
        io[name] = nc.declare_dram_parameter(name, shape, dt, isOutput=is_out).ap()
    return io


def build(cfg: Cfg, split: bool = True):
    nc = bass.Bass(num_devices=8)
    io = declare_io(nc, cfg)
    with tile.TileContext(nc) as tc:
        decoder_kernel(tc, cfg, io)
    if split:
        split_drain_waits(nc)
    return nc


# ======================= host-side prep =======================


def make_diag_masks(cfg: Cfg):
    """[128, 2*256] fp16 0/1: diag tile di covers keys k=di*128+p vs
    queries q: 1.0 iff di*128+p <= q (valid), else 0.0."""
    m = np.zeros((128, 512), np.float32)
    q = np.arange(256)[None, :]
    for di in range(2):
        kg = di * 128 + np.arange(128)[:, None]
        m[:, di * 256 : (di + 1) * 256] = (kg <= q).astype(np.float32)
    return m.astype(np.float16)


def host_prep(cfg: Cfg, x, ln1_g, ln1_b, w_qkv, w_proj, ln2_g, ln2_b, w_fc1, w_fc2):
    """Returns (in_maps list of 8 dicts, assemble(results)->full out)."""
    D, DH = cfg.D, cfg.DH
    ND, NFF = cfg.ND, cfg.NFF
    x = np.asarray(x, np.float32)
    w_qkv = np.asarray(w_qkv, np.float32)
    bqkv = np.asarray(ln1_b, np.float32) @ w_qkv
    w_qkv = w_qkv * np.asarray(ln1_g, np.float32)[:, None]
    s = 1.0 / np.sqrt(DH).astype(np.float32)
    bq_full = bqkv[0:D] * s
    bk_full = bqkv[D : 2 * D]
    bv_full = bqkv[2 * D : 3 * D]
    wq_full = w_qkv[:, 0:D] * s
    wk_full = w_qkv[:, D : 2 * D]
    wv_full = w_qkv[:, 2 * D : 3 * D]
    bfc1 = np.asarray(ln2_b, np.float32) @ np.asarray(w_fc1, np.float32)
    wfc1 = np.asarray(w_fc1, np.float32) * np.asarray(ln2_g, np.float32)[:, None]
    wfc2 = np.asarray(w_fc2, np.float32)

    wd = np.float32 if cfg.mmdt == "float32" else np.float16
    pack_kc = lambda w: np.ascontiguousarray(
        w.reshape(ND, 128, -1, 128).transpose(2, 1, 0, 3).astype(wd)
    )  # w[kt*128+p, o*128+c] -> [o, p, kt, c]
    shared = {
        "wproj": np.asarray(w_proj, np.float32).astype(wd),
        "wfc1": pack_kc(wfc1),
        "bfc1": bfc1.astype(np.float32),
        "wfc2": np.ascontiguousarray(
            wfc2.reshape(NFF, 128, 2, 512).transpose(2, 0, 1, 3).astype(wd)
        ),
        "maskd": make_diag_masks(cfg),
    }

    in_maps = []
    for c in range(8):
        fl, fh = c * 128, (c + 1) * 128  # head-pair feature slice
        # core c owns the 256-token quarter [256c, 256c+256) of EACH batch
        im = dict(shared)
        im["x_fm"] = np.ascontiguousarray(
            np.concatenate([x[0].T, x[1].T], axis=1).astype(wd)
        )
        im["xq32"] = np.ascontiguousarray(
            np.concatenate(
                [x[0][c * 256 : (c + 1) * 256], x[1][c * 256 : (c + 1) * 256]],
                axis=0,
            ).T
        )
        im["wq"] = pack_kc(wq_full[:, fl:fh])[0]
        im["wk"] = pack_kc(wk_full[:, fl:fh])[0]
        im["wv"] = pack_kc(wv_full[:, fl:fh])[0]
        im["bq"] = bq_full[fl:fh].astype(np.float32)
        im["bk"] = bk_full[fl:fh].astype(np.float32)
        im["vb"] = bv_full[fl:fh].astype(np.float32)
        im["ckn"] = (-wk_full[:, fl:fh].sum(axis=0))[None, :].astype(wd)
        im["cqn"] = (-wq_full[:, fl:fh].sum(axis=0))[None, :].astype(wd)
        im["cvn"] = (-wv_full[:, fl:fh].sum(axis=0))[None, :].astype(wd)
        in_maps.append(im)

    def assemble(results):
        out = np.zeros((cfg.B, cfg.T, D), np.float32)
        for c in range(8):
            o = results[c]["out"].T
            out[0][c * 256 : (c + 1) * 256] = o[0:256]
            out[1][c * 256 : (c + 1) * 256] = o[256:512]
        return out

    return in_maps, assemble


# ======================= public entry point =======================

LAST_RESULTS = {}
_CACHE = {}


def kernel(x, ln1_g, ln1_b, w_qkv, w_proj, ln2_g, ln2_b, w_fc1, w_fc2,
           _trace=False):
    """Full-input decoder block on 8 TRN2 NeuronCores; returns full output."""
    from concourse.bass_utils import run_bass_kernel_spmd

    cfg = Cfg()
    in_maps, assemble = host_prep(
        cfg, x, ln1_g, ln1_b, w_qkv, w_proj, ln2_g, ln2_b, w_fc1, w_fc2
    )
    if "nc" not in _CACHE:
        _CACHE["nc"] = build(cfg)
    res = run_bass_kernel_spmd(
        _CACHE["nc"], in_maps, core_ids=list(range(8)), trace=_trace
    )
    LAST_RESULTS["res"] = res
    return assemble(res.results)
